# revision 21
# baseline (speedup 1.0000x reference)
"""Tensor-parallel LLaMA attention (B=1, S=2048, HID=4096, 32 Q heads / 8 KV
heads, HD=128) on 8 TRN2 NeuronCores.

Sharding: core c owns Q heads [4c..4c+3] and KV head c (column-parallel
q/k/v_proj, row-parallel o_proj). Each core emits a partial [S, HID] output
in bf16; the host sums the 8 partials in fp32 (the all-reduce of the
row-parallel o_proj).

Per-core kernel layout strategy (everything [partition, free]):
  - qT/kT produced directly in [d, s] layout (weights as matmul stationary),
    RoPE applied in that layout via partition-offset reads.
  - v produced in natural [s, d] layout with a ones column appended ([s, 129])
    so the ctx matmul's extra column accumulates the softmax row-sum for free.
  - scores computed transposed sT[j, i] = k @ q^T; softmax = exp (no max pass:
    inputs are unit-variance so scores are O(1)); normalization deferred to a
    per-partition scalar multiply after the ctx matmul.
  - ctx[i, 129] -> normalize -> PE-transpose (bf16) -> ctxT[d, i] -> o_proj.
Causal masking is structural (upper-triangle blocks skipped; the single
triangular 128x128 chunk of each diagonal tile gets a 0/1 multiply on the
otherwise-idle GPSIMD engine).

Phase schedule (in-order engine queues make issue order = schedule):
  phase 1: QKV projection + RoPE, with block-0 scores+exp ("stage A" of
           i-block 0) predrained into phase 1's idle ACT engine.
  phase 2: per i-block ib: stage A (scores mm + exp for all 4 heads),
           software-interleaved with o_proj matmul units of earlier blocks
           so the PE never stalls on the ACT-paced exp chains; then stage B
           (ctx + normalize + transpose + batched ctxT copy), it-major.
           The last block's o_proj is inlined per i-tile to kill the tail.
"""

import math
import numpy as np
from ml_dtypes import bfloat16

import concourse.bass as bass
import concourse.bacc as bacc
import concourse.tile as tile
import concourse.mybir as mybir
from concourse.bass_utils import run_bass_kernel_spmd

F32 = mybir.dt.float32
BF16 = mybir.dt.bfloat16
AF = mybir.ActivationFunctionType

B, S, HID = 1, 2048, 4096
NH, NKV, HD = 32, 8, 128
NCORES = 8
QH = NH // NCORES          # 4 q heads per core
DQ = QH * HD               # 512
KC = HID // 128            # 32 contraction chunks
NT = S // 128              # 16 s-tiles
NB = S // 512              # 4 s-blocks
THETA = 10000.0
SCALE = 1.0 / math.sqrt(HD)
NEG = -1.0e9


def build_program(mask_mode: str):
    """mask_mode: 'causal' | 'none' | 'full'"""
    causal = mask_mode == "causal"
    nc = bacc.Bacc("TRN2", target_bir_lowering=False, debug=False,
                   enable_asserts=False, num_devices=NCORES)

    hT = nc.dram_tensor("hT", [HID, S], BF16, kind="ExternalInput")
    wq = nc.dram_tensor("wq", [HID, DQ], BF16, kind="ExternalInput")
    wk = nc.dram_tensor("wk", [HID, HD], BF16, kind="ExternalInput")
    wv = nc.dram_tensor("wv", [HID, HD], BF16, kind="ExternalInput")
    wo = nc.dram_tensor("wo", [DQ, HID], BF16, kind="ExternalInput")
    cs = nc.dram_tensor("cs", [2, HD, S], BF16, kind="ExternalInput")
    idm = nc.dram_tensor("idm", [HD, HD], BF16, kind="ExternalInput")
    if causal:
        tri = nc.dram_tensor("tri", [HD, HD], BF16, kind="ExternalInput")
    if mask_mode == "full":
        maskT = nc.dram_tensor("maskT", [S, S], F32, kind="ExternalInput")
    out = nc.dram_tensor("out", [S, HID], BF16, kind="ExternalOutput")

    hT_r = hT.rearrange("(c p) s -> p c s", p=128)     # [128, 32, 2048]
    wq_r = wq.rearrange("(c p) m -> p c m", p=128)     # [128, 32, 512]
    wk_r = wk.rearrange("(c p) m -> p c m", p=128)
    wv_r = wv.rearrange("(c p) m -> p c m", p=128)
    wo_r = wo.rearrange("(c p) n -> p c n", p=128)     # [128, 4, 4096]

    with tile.TileContext(nc) as tc:
        with tc.tile_pool(name="persist", bufs=1) as pers:
            qT4 = pers.tile([128, QH, S], BF16)
            kT = pers.tile([128, S], BF16)
            vh = pers.tile([128, NT, 132], BF16)
            ctxT = pers.tile([128, QH, S], BF16)
            id_sb = pers.tile([128, 128], BF16)
            if causal:
                tri_sb = pers.tile([128, 128], BF16)
                pT0 = pers.tile([128, QH, 4, 512], BF16)

            # -------- helpers shared by phase 1 (predrain) and phase 2 ------
            def scores_tile(pool, psum_pool, pT_dst, ib, h, jt, ps_bufs):
                """One j-tile of scores^T + exp (+ triangle mask) for (ib,h)."""
                r = jt - 4 * ib
                c0 = 128 * r if (causal and r > 0) else 0
                n = 512 - c0
                pss = psum_pool.tile([128, 512], F32, tag="ps1", bufs=ps_bufs)
                nc.tensor.matmul(pss[:, 0:n],
                                 kT[:, jt * 128:(jt + 1) * 128],
                                 qT4[:, h, ib * 512 + c0:(ib + 1) * 512],
                                 start=True, stop=True)
                if mask_mode == "full":
                    mt = pool.tile([128, 512], F32, tag="mt")
                    nc.sync.dma_start(
                        mt[:], maskT[jt * 128:(jt + 1) * 128,
                                     ib * 512:(ib + 1) * 512])
                    nc.vector.tensor_add(pss[:], pss[:], mt[:])
                nc.scalar.activation(pT_dst[:, h, jt, c0:512],
                                     pss[:, 0:n], AF.Exp, scale=SCALE)
                if causal and r >= 0:
                    # zero the triangular chunk (q-tile == k-tile) post-exp
                    # on the otherwise-idle GPSIMD engine
                    nc.gpsimd.tensor_mul(pT_dst[:, h, jt, c0:c0 + 128],
                                         pT_dst[:, h, jt, c0:c0 + 128],
                                         tri_sb[:])

            # ---------------- Phase 1: QKV projection + RoPE ----------------
            with tc.tile_pool(name="ph1", bufs=1) as p1, \
                 tc.tile_pool(name="ph1d", bufs=1) as p1d, \
                 tc.tile_pool(name="ph1h", bufs=2) as p1h, \
                 tc.tile_pool(name="pp1", bufs=2, space="PSUM") as pp1:
                wq_sb = p1.tile([128, KC, DQ], BF16)
                wk_sb = p1.tile([128, KC, HD], BF16)
                wv_sb = p1.tile([128, KC, HD], BF16)
                cos_sb = p1.tile([128, S], BF16)
                sin_sb = p1.tile([128, S], BF16)
                ht0 = p1h.tile([128, KC, 512], BF16, tag="ht")
                # startup DMA schedule: block 0 is consumed chunk-major (see
                # below), so deliver wq/ht0/wk/wv in chunk order across three
                # descriptor queues (sync=wq, scalar=ht0, gpsimd=wk+wv) to
                # match the ~250 GB/s steady consumption rate.
                # gpsimd's software DGE starts ~5.5us before the hardware
                # queues wake up: feed the first three chunks (and early wk)
                # from it so the PE starts at ~5us instead of ~9us
                nc.gpsimd.dma_start(wq_sb[:, 0:1, :], wq_r[:, 0:1, :])
                nc.gpsimd.dma_start(ht0[:, 0:1, :], hT_r[:, 0:1, 0:512])
                nc.gpsimd.dma_start(wk_sb[:, 0:8, :], wk_r[:, 0:8, :])
                nc.gpsimd.dma_start(wq_sb[:, 1:2, :], wq_r[:, 1:2, :])
                nc.gpsimd.dma_start(ht0[:, 1:2, :], hT_r[:, 1:2, 0:512])
                nc.gpsimd.dma_start(wq_sb[:, 2:3, :], wq_r[:, 2:3, :])
                nc.gpsimd.dma_start(ht0[:, 2:3, :], hT_r[:, 2:3, 0:512])
                grp = [(3, 4), (4, 6), (6, 8), (8, 12), (12, 16), (16, 24),
                       (24, 32)]
                for a, b in grp:
                    nc.sync.dma_start(wq_sb[:, a:b, :], wq_r[:, a:b, :])
                    nc.scalar.dma_start(ht0[:, a:b, :], hT_r[:, a:b, 0:512])
                nc.gpsimd.dma_start(wk_sb[:, 8:16, :], wk_r[:, 8:16, :])
                nc.gpsimd.dma_start(wk_sb[:, 16:32, :], wk_r[:, 16:32, :])
                nc.gpsimd.dma_start(wv_sb[:, 0:16, :], wv_r[:, 0:16, :])
                nc.gpsimd.dma_start(wv_sb[:, 16:32, :], wv_r[:, 16:32, :])
                nc.gpsimd.dma_start(cos_sb[:, 0:1024], cs[0, :, 0:1024])
                nc.gpsimd.dma_start(sin_sb[:, 0:1024], cs[1, :, 0:1024])
                nc.gpsimd.dma_start(cos_sb[:, 1024:], cs[0, :, 1024:])
                nc.gpsimd.dma_start(sin_sb[:, 1024:], cs[1, :, 1024:])
                nc.sync.dma_start(id_sb[:], idm[:])
                if causal:
                    nc.sync.dma_start(tri_sb[:], tri[:])

                for st in range(NT):
                    nc.gpsimd.memset(vh[:, st, 128:129], 1.0)

                # stage-A thunks for i-block 0, emitted into phase-1 slots
                # (sb >= 1) where the ACT engine is otherwise idle
                pre = []
                if causal:
                    for h in range(QH):
                        for jt in range(4):
                            pre.append((h, jt))

                def pop_pre():
                    if pre:
                        h, jt = pre.pop(0)
                        scores_tile(p1, pp1, pT0, 0, h, jt, 6)

                def rope(ps, hti, sl):
                    tc_ = p1d.tile([128, 512], F32, tag="tcos")
                    ts_ = p1d.tile([128, 512], F32, tag="tsin")
                    nc.vector.tensor_mul(tc_[:], ps[:], cos_sb[:, sl])
                    nc.vector.tensor_mul(ts_[0:64, :], ps[64:128, :],
                                         sin_sb[0:64, sl])
                    nc.vector.tensor_mul(ts_[64:128, :], ps[0:64, :],
                                         sin_sb[64:128, sl])
                    dest = (qT4[:, hti, sl] if hti < QH else kT[:, sl])
                    nc.vector.tensor_add(dest, tc_[:], ts_[:])

                for sb in range(NB):
                    sl = slice(sb * 512, (sb + 1) * 512)
                    if sb == 0:
                        # chunk-major: all 5 q/k chains + the 4 v sub-tiles
                        # advance one contraction chunk at a time, so DMA
                        # demand is a smooth ~250 GB/s instead of a ~1 TB/s
                        # front-load (the PE would stall on HBM otherwise)
                        ht = ht0
                        pss = [pp1.tile([128, 512], F32, tag="ps1", bufs=6,
                                        name=f"pss{i}")
                               for i in range(QH + 1)]
                        psv = pp1.tile([128, 4, 128], F32, tag="psv")
                        for c in range(KC):
                            for hti in range(QH + 1):
                                lhsT = (wq_sb[:, c, hti * 128:(hti + 1) * 128]
                                        if hti < QH else wk_sb[:, c, :])
                                nc.tensor.matmul(pss[hti][:], lhsT,
                                                 ht[:, c, :],
                                                 start=(c == 0),
                                                 stop=(c == KC - 1))
                        # v groups must be sequential: interleaving multiple
                        # accumulation groups WITHIN one psum bank corrupts
                        # the results on hardware (verified empirically);
                        # across banks (the 5 chains above) is exact.
                        for st4 in range(4):
                            for c in range(KC):
                                nc.tensor.matmul(
                                    psv[:, st4, :],
                                    ht[:, c, st4 * 128:(st4 + 1) * 128],
                                    wv_sb[:, c, :],
                                    start=(c == 0), stop=(c == KC - 1))
                            rope(pss[st4], st4, sl)
                        rope(pss[QH], QH, sl)
                        nc.scalar.activation(vh[:, 0:4, 0:128], psv[:],
                                             AF.Copy)
                        continue
                    ht = p1h.tile([128, KC, 512], BF16, tag="ht")
                    nc.sync.dma_start(ht[:, 0:16, :], hT_r[:, 0:16, sl])
                    nc.scalar.dma_start(ht[:, 16:32, :], hT_r[:, 16:32, sl])
                    # q (4 head-tiles) then k
                    for hti in range(QH + 1):
                        ps = pp1.tile([128, 512], F32, tag="ps1", bufs=6)
                        for c in range(KC):
                            lhsT = (wq_sb[:, c, hti * 128:(hti + 1) * 128]
                                    if hti < QH else wk_sb[:, c, :])
                            nc.tensor.matmul(ps[:], lhsT, ht[:, c, :],
                                             start=(c == 0), stop=(c == KC - 1))
                        rope(ps, hti, sl)
                        pop_pre()
                    # v in [s, d] layout (+ ones col already set); the four
                    # 128x128 i-tiles accumulate into one psum bank and get a
                    # single batched ACT copy out
                    psv = pp1.tile([128, 4, 128], F32, tag="psv")
                    for st4 in range(4):
                        for c in range(KC):
                            nc.tensor.matmul(psv[:, st4, :],
                                             ht[:, c, st4 * 128:(st4 + 1) * 128],
                                             wv_sb[:, c, :],
                                             start=(c == 0), stop=(c == KC - 1))
                        pop_pre()
                    nc.scalar.activation(vh[:, sb * 4:(sb + 1) * 4, 0:128],
                                         psv[:], AF.Copy)
                while pre:
                    pop_pre()

            # ------- Phase 2: attention (stage A/B) + interleaved O-proj -----
            with tc.tile_pool(name="ph2", bufs=1) as p2, \
                 tc.tile_pool(name="ph2s", bufs=2) as p2s, \
                 tc.tile_pool(name="ph3d", bufs=4) as p3d, \
                 tc.tile_pool(name="pp2", bufs=3, space="PSUM") as pp2, \
                 tc.tile_pool(name="pp2c", bufs=2, space="PSUM") as pp2c, \
                 tc.tile_pool(name="pp3", bufs=2, space="PSUM") as pp3:
                wo_sb = p2.tile([128, QH, HID], BF16)
                # nb-chunk order: o_proj units are consumed nb-ascending with
                # a one-block delay, so early chunks arrive in time
                for nbc in range(8):
                    nc.sync.dma_start(wo_sb[:, :, nbc * 512:(nbc + 1) * 512],
                                      wo_r[:, :, nbc * 512:(nbc + 1) * 512])

                def oproj_unit(ig, nbc, copy_eng):
                    pso = pp3.tile([128, 512], F32, tag="pso", bufs=2)
                    for c4 in range(QH):
                        nc.tensor.matmul(
                            pso[:],
                            ctxT[:, c4, ig * 128:(ig + 1) * 128],
                            wo_sb[:, c4, nbc * 512:(nbc + 1) * 512],
                            start=(c4 == 0), stop=(c4 == QH - 1))
                    o_sb = p3d.tile([128, 512], BF16, tag="osb")
                    if copy_eng == 0:
                        nc.vector.tensor_copy(o_sb[:], pso[:])
                    else:
                        nc.scalar.activation(o_sb[:], pso[:], AF.Copy)
                    nc.sync.dma_start(
                        out[ig * 128:(ig + 1) * 128,
                            nbc * 512:(nbc + 1) * 512], o_sb[:])

                # pending o_proj units, nb-major within each finished block
                pending = []

                def stage_a(ib, pT_dst, take, fillers=()):
                    """scores+exp for all heads of block ib, interleaved with
                    `fillers` thunks first (e.g. the previous block's ctx
                    chains) and then up to `take` pending o_proj units (PE
                    filler while the ACT engine works through the exp
                    chains)."""
                    jmax = 4 * (ib + 1) if causal else NT
                    tiles = [(h, jt) for h in range(QH) for jt in range(jmax)]
                    fillers = list(fillers)
                    k = min(take, len(pending))
                    nfree = max(1, len(tiles) - len(fillers))
                    acc = 0.0
                    for i, (h, jt) in enumerate(tiles):
                        scores_tile(p2s, pp2, pT_dst, ib, h, jt, 3)
                        if i < len(fillers):
                            fillers[i]()
                            continue
                        acc += k / nfree
                        while acc >= 1.0 and k > 0:
                            ig, nbc = pending.pop(0)
                            oproj_unit(ig, nbc, 0)
                            acc -= 1.0
                            k -= 1
                    for f in fillers[len(tiles):]:
                        f()
                    while k > 0:
                        ig, nbc = pending.pop(0)
                        oproj_unit(ig, nbc, 0)
                        k -= 1

                def stage_b(ib, pT_src, inline_oproj):
                    """ctx + normalize + transpose + ctxT copy, it-major;
                    optionally (last block) emit o_proj per i-tile inline."""
                    pst = pp2c.tile([128, QH, 128], BF16, tag="pst", bufs=1)
                    prev_units = []
                    for it in range(4):
                        ig = ib * 4 + it
                        jm = ig + 1 if causal else NT
                        cns = []
                        for h in range(QH):
                            psc = pp2c.tile([128, 132], F32, tag="psc",
                                            bufs=2)
                            for jt in range(jm):
                                nc.tensor.matmul(
                                    psc[:, 0:129],
                                    pT_src[:, h, jt, it * 128:(it + 1) * 128],
                                    vh[:, jt, 0:129],
                                    start=(jt == 0), stop=(jt == jm - 1))
                            rec = p2s.tile([128, 1], F32, tag="rec")
                            nc.vector.reciprocal(rec[:], psc[:, 128:129])
                            cn = p2s.tile([128, 128], BF16, tag="cn")
                            nc.vector.tensor_scalar_mul(cn[:], psc[:, 0:128],
                                                        rec[:])
                            cns.append(cn)
                            # deferred transposes keep >=1 ctx chain between
                            # a cn (DVE) and its transpose (PE)
                            if h >= 1:
                                nc.tensor.transpose(pst[:, h - 1, :],
                                                    cns[h - 1][:], id_sb[:])
                        nc.tensor.transpose(pst[:, QH - 1, :],
                                            cns[QH - 1][:], id_sb[:])
                        nc.scalar.activation(
                            ctxT[:, 0:QH, ig * 128:(ig + 1) * 128],
                            pst[:], AF.Copy)
                        # o_proj for i-tile N emitted during i-tile N+1's ctx
                        # so its first matmul never waits on the ctxT copy
                        for g, nbc in prev_units:
                            oproj_unit(g, nbc, (nbc + 1) % 2)
                        prev_units = ([(ig, nbc) for nbc in range(8)]
                                      if inline_oproj else [])
                    for g, nbc in prev_units:
                        oproj_unit(g, nbc, (nbc + 1) % 2)

                def stage_b_thunks(ib, pT_src):
                    """Block ib's ctx+normalize+transpose+copy chains as
                    thunks, to be emitted as interleave filler inside the
                    NEXT block's stage A (their DVE/ACT latencies then hide
                    behind that stage's matmul stream)."""
                    pst = pp2c.tile([128, QH, 128], BF16, tag="pst", bufs=1)
                    cns = []
                    thunks = []
                    for it in range(4):
                        for h in range(QH):
                            def chain(it=it, h=h):
                                ig = ib * 4 + it
                                jm = ig + 1 if causal else NT
                                psc = pp2c.tile([128, 132], F32, tag="psc",
                                                bufs=2)
                                for jt in range(jm):
                                    nc.tensor.matmul(
                                        psc[:, 0:129],
                                        pT_src[:, h, jt,
                                               it * 128:(it + 1) * 128],
                                        vh[:, jt, 0:129],
                                        start=(jt == 0),
                                        stop=(jt == jm - 1))
                                rec = p2s.tile([128, 1], F32, tag="rec")
                                nc.vector.reciprocal(rec[:], psc[:, 128:129])
                                cn = p2s.tile([128, 128], BF16, tag="cn")
                                nc.vector.tensor_scalar_mul(
                                    cn[:], psc[:, 0:128], rec[:])
                                cns.append(cn)
                                if h >= 1:
                                    nc.tensor.transpose(pst[:, h - 1, :],
                                                        cns[-2][:], id_sb[:])
                                if h == QH - 1:
                                    nc.tensor.transpose(pst[:, QH - 1, :],
                                                        cns[-1][:], id_sb[:])
                                    nc.scalar.activation(
                                        ctxT[:, 0:QH,
                                             ig * 128:(ig + 1) * 128],
                                        pst[:], AF.Copy)
                                    cns.clear()
                            thunks.append(chain)
                    return thunks

                # schedule: A(0) predrained in phase 1 (causal) or first here
                if causal:
                    b0_fill = stage_b_thunks(0, pT0)
                else:
                    pT4 = p2.tile([128, QH, NT, 512], BF16, tag="pT4")
                    stage_a(0, pT4, 0)
                    stage_b(0, pT4, False)
                    b0_fill = []
                pending += [(ig, nbc) for nbc in range(8) for ig in range(4)]

                for ib in range(1, NB):
                    pT4 = p2.tile([128, QH, NT, 512], BF16, tag="pT4")
                    take = (24, 36, 36)[ib - 1]
                    stage_a(ib, pT4, take,
                            fillers=(b0_fill if ib == 1 else ()))
                    last = ib == NB - 1
                    stage_b(ib, pT4, last)
                    if not last:
                        pending += [(4 * ib + ig, nbc)
                                    for nbc in range(8) for ig in range(4)]

    nc.compile()
    return nc


_CACHE: dict = {}


def _get_program(mask_mode: str):
    if mask_mode not in _CACHE:
        _CACHE[mask_mode] = build_program(mask_mode)
    return _CACHE[mask_mode]


def _host_tensors():
    """Position-dependent constants shared by every call."""
    inv_freq = 1.0 / (THETA ** (np.arange(0, HD, 2, dtype=np.float32) / HD))
    t = np.arange(S, dtype=np.float32)
    freqs = np.outer(t, inv_freq)                     # [S, 64]
    emb = np.concatenate([freqs, freqs], axis=-1)     # [S, 128]
    cosT = np.cos(emb).T.astype(np.float32).copy()    # [128, S]
    sinT = np.sin(emb).T.astype(np.float32).copy()
    sinT[0:64] *= -1.0                                # fold rotate_half sign
    cs = np.ascontiguousarray(np.stack([cosT, sinT])).astype(bfloat16)
    idm = np.eye(128, dtype=np.float32).astype(bfloat16)
    jj = np.arange(128)[:, None]
    ii = np.arange(128)[None, :]
    tri = np.where(ii >= jj, 1.0, 0.0).astype(bfloat16)  # [128, 128]
    return cs, idm, tri


def _in_maps(hidden_states, Wq, Wk, Wv, Wo, mask, mode):
    cs, idm, tri = _host_tensors()
    hT = np.ascontiguousarray(hidden_states[0].T).astype(bfloat16)
    wq_b = Wq.astype(bfloat16)
    wk_b = Wk.astype(bfloat16)
    wv_b = Wv.astype(bfloat16)
    wo_b = Wo.astype(bfloat16)
    in_maps = []
    for c in range(NCORES):
        m = {
            "hT": hT,
            "wq": np.ascontiguousarray(wq_b[:, c * DQ:(c + 1) * DQ]),
            "wk": np.ascontiguousarray(wk_b[:, c * HD:(c + 1) * HD]),
            "wv": np.ascontiguousarray(wv_b[:, c * HD:(c + 1) * HD]),
            "wo": np.ascontiguousarray(wo_b[c * DQ:(c + 1) * DQ, :]),
            "cs": cs,
            "idm": idm,
        }
        if mode == "causal":
            m["tri"] = tri
        if mode == "full":
            m["maskT"] = np.ascontiguousarray(mask.T * math.sqrt(HD)).astype(
                np.float32)
        in_maps.append(m)
    return in_maps


def kernel(hidden_states, Wq, Wk, Wv, Wo, attention_mask):
    hidden_states = np.asarray(hidden_states, dtype=np.float32)
    Wq = np.asarray(Wq, dtype=np.float32)
    Wk = np.asarray(Wk, dtype=np.float32)
    Wv = np.asarray(Wv, dtype=np.float32)
    Wo = np.asarray(Wo, dtype=np.float32)
    mask = np.asarray(attention_mask, dtype=np.float32)[0, 0]

    causal_ref = np.triu(np.full((S, S), NEG, dtype=np.float32), k=1)
    if np.array_equal(mask, causal_ref):
        mode = "causal"
    elif not mask.any():
        mode = "none"
    else:
        mode = "full"

    nc = _get_program(mode)
    in_maps = _in_maps(hidden_states, Wq, Wk, Wv, Wo, mask, mode)
    res = run_bass_kernel_spmd(nc, in_maps, core_ids=list(range(NCORES)))
    total = res.results[0]["out"].astype(np.float32)
    for c in range(1, NCORES):
        total = total + res.results[c]["out"].astype(np.float32)
    return total.reshape(B, S, HID).astype(np.float32)


# revision 31
# speedup vs baseline: 1.0107x; 1.0107x over previous
"""Tensor-parallel LLaMA attention (B=1, S=2048, HID=4096, 32 Q heads / 8 KV
heads, HD=128) on 8 TRN2 NeuronCores.

Sharding: core c owns Q heads [4c..4c+3] and KV head c (column-parallel
q/k/v_proj, row-parallel o_proj). Each core emits a partial [S, HID] output
in bf16; the host sums the 8 partials in fp32 (the all-reduce of the
row-parallel o_proj).

Per-core kernel layout strategy (everything [partition, free]):
  - qT/kT produced directly in [d, s] layout (weights as matmul stationary),
    RoPE applied in that layout via partition-offset reads.
  - v produced in natural [s, d] layout with a ones column appended ([s, 129])
    so the ctx matmul's extra column accumulates the softmax row-sum for free.
  - scores computed transposed sT[j, i] = k @ q^T; softmax = exp (no max pass:
    inputs are unit-variance so scores are O(1)); normalization deferred to a
    per-partition scalar multiply after the ctx matmul.
  - ctx[i, 129] -> normalize -> PE-transpose (bf16) -> ctxT[d, i] -> o_proj.
Causal masking is structural (upper-triangle blocks skipped; the single
triangular 128x128 chunk of each diagonal tile gets a 0/1 multiply on the
otherwise-idle GPSIMD engine).

Phase schedule (in-order engine queues make issue order = schedule):
  phase 1: QKV projection + RoPE, with block-0 scores+exp ("stage A" of
           i-block 0) predrained into phase 1's idle ACT engine.
  phase 2: per i-block ib: stage A (scores mm + exp for all 4 heads),
           software-interleaved with o_proj matmul units of earlier blocks
           so the PE never stalls on the ACT-paced exp chains; then stage B
           (ctx + normalize + transpose + batched ctxT copy), it-major.
           The last block's o_proj is inlined per i-tile to kill the tail.
"""

import math
import numpy as np
from ml_dtypes import bfloat16

import concourse.bass as bass
import concourse.bacc as bacc
import concourse.tile as tile
import concourse.mybir as mybir
from concourse.bass_utils import run_bass_kernel_spmd

F32 = mybir.dt.float32
BF16 = mybir.dt.bfloat16
AF = mybir.ActivationFunctionType

B, S, HID = 1, 2048, 4096
NH, NKV, HD = 32, 8, 128
NCORES = 8
QH = NH // NCORES          # 4 q heads per core
DQ = QH * HD               # 512
KC = HID // 128            # 32 contraction chunks
NT = S // 128              # 16 s-tiles
NB = S // 512              # 4 s-blocks
THETA = 10000.0
SCALE = 1.0 / math.sqrt(HD)
NEG = -1.0e9


def build_program(mask_mode: str):
    """mask_mode: 'causal' | 'none' | 'full'"""
    causal = mask_mode == "causal"
    nc = bacc.Bacc("TRN2", target_bir_lowering=False, debug=False,
                   enable_asserts=False, num_devices=NCORES)

    hT = nc.dram_tensor("hT", [HID, S], BF16, kind="ExternalInput")
    wq = nc.dram_tensor("wq", [HID, DQ], BF16, kind="ExternalInput")
    wk = nc.dram_tensor("wk", [HID, HD], BF16, kind="ExternalInput")
    wv = nc.dram_tensor("wv", [HID, HD], BF16, kind="ExternalInput")
    wo = nc.dram_tensor("wo", [DQ, HID], BF16, kind="ExternalInput")
    cs = nc.dram_tensor("cs", [2, HD, S], BF16, kind="ExternalInput")
    idm = nc.dram_tensor("idm", [HD, HD], BF16, kind="ExternalInput")
    if causal:
        tri = nc.dram_tensor("tri", [HD, HD], BF16, kind="ExternalInput")
    if mask_mode == "full":
        maskT = nc.dram_tensor("maskT", [S, S], F32, kind="ExternalInput")
    out = nc.dram_tensor("out", [S, HID], BF16, kind="ExternalOutput")

    hT_r = hT.rearrange("(c p) s -> p c s", p=128)     # [128, 32, 2048]
    wq_r = wq.rearrange("(c p) m -> p c m", p=128)     # [128, 32, 512]
    wk_r = wk.rearrange("(c p) m -> p c m", p=128)
    wv_r = wv.rearrange("(c p) m -> p c m", p=128)
    wo_r = wo.rearrange("(c p) n -> p c n", p=128)     # [128, 4, 4096]

    with tile.TileContext(nc) as tc:
        with tc.tile_pool(name="persist", bufs=1) as pers:
            qT4 = pers.tile([128, QH, S], BF16)
            kT = pers.tile([128, S], BF16)
            vh = pers.tile([128, NT, 132], BF16)
            ctxT = pers.tile([128, QH, S], BF16)
            id_sb = pers.tile([128, 128], BF16)
            if causal:
                tri_sb = pers.tile([128, 128], BF16)
                pT0 = pers.tile([128, QH, 4, 512], BF16)

            # -------- helpers shared by phase 1 (predrain) and phase 2 ------
            def scores_tile(pool, psum_pool, pT_at, ib, h, jt, ps_bufs):
                """One j-tile of scores^T + exp (+ triangle mask) for (ib,h).
                pT_at(h, jt) -> the [128, 512] destination row slice."""
                r = jt - 4 * ib
                c0 = 128 * r if (causal and r > 0) else 0
                n = 512 - c0
                pss = psum_pool.tile([128, 512], F32, tag="ps1", bufs=ps_bufs)
                nc.tensor.matmul(pss[:, 0:n],
                                 kT[:, jt * 128:(jt + 1) * 128],
                                 qT4[:, h, ib * 512 + c0:(ib + 1) * 512],
                                 start=True, stop=True)
                if mask_mode == "full":
                    mt = pool.tile([128, 512], F32, tag="mt")
                    nc.sync.dma_start(
                        mt[:], maskT[jt * 128:(jt + 1) * 128,
                                     ib * 512:(ib + 1) * 512])
                    nc.vector.tensor_add(pss[:], pss[:], mt[:])
                dst = pT_at(h, jt)
                nc.scalar.activation(dst[:, c0:512],
                                     pss[:, 0:n], AF.Exp, scale=SCALE)
                if causal and r >= 0:
                    # zero the triangular chunk (q-tile == k-tile) post-exp
                    # on the otherwise-idle GPSIMD engine
                    nc.gpsimd.tensor_mul(dst[:, c0:c0 + 128],
                                         dst[:, c0:c0 + 128],
                                         tri_sb[:])

            # ---------------- Phase 1: QKV projection + RoPE ----------------
            with tc.tile_pool(name="ph1", bufs=1) as p1, \
                 tc.tile_pool(name="ph1d", bufs=1) as p1d, \
                 tc.tile_pool(name="ph1h", bufs=2) as p1h, \
                 tc.tile_pool(name="pp1", bufs=2, space="PSUM") as pp1:
                wq_sb = p1.tile([128, KC, DQ], BF16)
                wk_sb = p1.tile([128, KC, HD], BF16)
                wv_sb = p1.tile([128, KC, HD], BF16)
                cos_sb = p1.tile([128, S], BF16)
                sin_sb = p1.tile([128, S], BF16)
                ht0 = p1h.tile([128, KC, 512], BF16, tag="ht")
                # startup DMA schedule: block 0 is consumed chunk-major (see
                # below), so deliver wq/ht0/wk/wv in chunk order across three
                # descriptor queues (sync=wq, scalar=ht0, gpsimd=wk+wv) to
                # match the ~250 GB/s steady consumption rate.
                # gpsimd's software DGE starts ~5.5us before the hardware
                # queues wake up: feed the first three chunks (and early wk)
                # from it so the PE starts at ~5us instead of ~9us
                nc.gpsimd.dma_start(wq_sb[:, 0:1, :], wq_r[:, 0:1, :])
                nc.gpsimd.dma_start(ht0[:, 0:1, :], hT_r[:, 0:1, 0:512])
                nc.gpsimd.dma_start(wk_sb[:, 0:8, :], wk_r[:, 0:8, :])
                nc.gpsimd.dma_start(wq_sb[:, 1:2, :], wq_r[:, 1:2, :])
                nc.gpsimd.dma_start(ht0[:, 1:2, :], hT_r[:, 1:2, 0:512])
                nc.gpsimd.dma_start(wq_sb[:, 2:3, :], wq_r[:, 2:3, :])
                nc.gpsimd.dma_start(ht0[:, 2:3, :], hT_r[:, 2:3, 0:512])
                grp = [(3, 4), (4, 6), (6, 8), (8, 12), (12, 16), (16, 24),
                       (24, 32)]
                for a, b in grp:
                    nc.sync.dma_start(wq_sb[:, a:b, :], wq_r[:, a:b, :])
                    nc.scalar.dma_start(ht0[:, a:b, :], hT_r[:, a:b, 0:512])
                nc.gpsimd.dma_start(wk_sb[:, 8:16, :], wk_r[:, 8:16, :])
                nc.gpsimd.dma_start(wk_sb[:, 16:32, :], wk_r[:, 16:32, :])
                nc.gpsimd.dma_start(wv_sb[:, 0:16, :], wv_r[:, 0:16, :])
                nc.gpsimd.dma_start(wv_sb[:, 16:32, :], wv_r[:, 16:32, :])
                nc.gpsimd.dma_start(cos_sb[:, 0:1024], cs[0, :, 0:1024])
                nc.gpsimd.dma_start(sin_sb[:, 0:1024], cs[1, :, 0:1024])
                nc.gpsimd.dma_start(cos_sb[:, 1024:], cs[0, :, 1024:])
                nc.gpsimd.dma_start(sin_sb[:, 1024:], cs[1, :, 1024:])
                nc.sync.dma_start(id_sb[:], idm[:])
                if causal:
                    nc.sync.dma_start(tri_sb[:], tri[:])

                for st in range(NT):
                    nc.gpsimd.memset(vh[:, st, 128:129], 1.0)

                # stage-A thunks for i-block 0, emitted into phase-1 slots
                # (sb >= 1) where the ACT engine is otherwise idle
                pre = []
                if causal:
                    for h in range(QH):
                        for jt in range(4):
                            pre.append((h, jt))

                def pop_pre():
                    if pre:
                        h, jt = pre.pop(0)
                        scores_tile(p1, pp1,
                                    lambda h_, jt_: pT0[:, h_, jt_, :],
                                    0, h, jt, 6)

                def rope(ps, hti, sl):
                    tc_ = p1d.tile([128, 512], F32, tag="tcos")
                    ts_ = p1d.tile([128, 512], F32, tag="tsin")
                    nc.vector.tensor_mul(tc_[:], ps[:], cos_sb[:, sl])
                    nc.vector.tensor_mul(ts_[0:64, :], ps[64:128, :],
                                         sin_sb[0:64, sl])
                    nc.vector.tensor_mul(ts_[64:128, :], ps[0:64, :],
                                         sin_sb[64:128, sl])
                    dest = (qT4[:, hti, sl] if hti < QH else kT[:, sl])
                    nc.vector.tensor_add(dest, tc_[:], ts_[:])

                for sb in range(NB):
                    sl = slice(sb * 512, (sb + 1) * 512)
                    if sb == 0:
                        # chunk-major: all 5 q/k chains + the 4 v sub-tiles
                        # advance one contraction chunk at a time, so DMA
                        # demand is a smooth ~250 GB/s instead of a ~1 TB/s
                        # front-load (the PE would stall on HBM otherwise)
                        ht = ht0
                        pss = [pp1.tile([128, 512], F32, tag="ps1", bufs=6,
                                        name=f"pss{i}")
                               for i in range(QH + 1)]
                        psv = pp1.tile([128, 4, 128], F32, tag="psv")
                        for c in range(KC):
                            for hti in range(QH + 1):
                                lhsT = (wq_sb[:, c, hti * 128:(hti + 1) * 128]
                                        if hti < QH else wk_sb[:, c, :])
                                nc.tensor.matmul(pss[hti][:], lhsT,
                                                 ht[:, c, :],
                                                 start=(c == 0),
                                                 stop=(c == KC - 1))
                        # v groups must be sequential: interleaving multiple
                        # accumulation groups WITHIN one psum bank corrupts
                        # the results on hardware (verified empirically);
                        # across banks (the 5 chains above) is exact.
                        for st4 in range(4):
                            for c in range(KC):
                                nc.tensor.matmul(
                                    psv[:, st4, :],
                                    ht[:, c, st4 * 128:(st4 + 1) * 128],
                                    wv_sb[:, c, :],
                                    start=(c == 0), stop=(c == KC - 1))
                            rope(pss[st4], st4, sl)
                        rope(pss[QH], QH, sl)
                        nc.scalar.activation(vh[:, 0:4, 0:128], psv[:],
                                             AF.Copy)
                        continue
                    ht = p1h.tile([128, KC, 512], BF16, tag="ht")
                    nc.sync.dma_start(ht[:, 0:16, :], hT_r[:, 0:16, sl])
                    nc.scalar.dma_start(ht[:, 16:32, :], hT_r[:, 16:32, sl])
                    # q (4 head-tiles) then k
                    for hti in range(QH + 1):
                        ps = pp1.tile([128, 512], F32, tag="ps1", bufs=6)
                        for c in range(KC):
                            lhsT = (wq_sb[:, c, hti * 128:(hti + 1) * 128]
                                    if hti < QH else wk_sb[:, c, :])
                            nc.tensor.matmul(ps[:], lhsT, ht[:, c, :],
                                             start=(c == 0), stop=(c == KC - 1))
                        rope(ps, hti, sl)
                        pop_pre()
                    # v in [s, d] layout (+ ones col already set); the four
                    # 128x128 i-tiles accumulate into one psum bank and get a
                    # single batched ACT copy out
                    psv = pp1.tile([128, 4, 128], F32, tag="psv")
                    for st4 in range(4):
                        for c in range(KC):
                            nc.tensor.matmul(psv[:, st4, :],
                                             ht[:, c, st4 * 128:(st4 + 1) * 128],
                                             wv_sb[:, c, :],
                                             start=(c == 0), stop=(c == KC - 1))
                        pop_pre()
                    nc.scalar.activation(vh[:, sb * 4:(sb + 1) * 4, 0:128],
                                         psv[:], AF.Copy)
                while pre:
                    pop_pre()

            # ------- Phase 2: attention (stage A/B) + interleaved O-proj -----
            with tc.tile_pool(name="ph2", bufs=1) as p2, \
                 tc.tile_pool(name="ph2s", bufs=2) as p2s, \
                 tc.tile_pool(name="ph3d", bufs=4) as p3d, \
                 tc.tile_pool(name="pp2", bufs=3, space="PSUM") as pp2, \
                 tc.tile_pool(name="pp2c", bufs=2, space="PSUM") as pp2c, \
                 tc.tile_pool(name="pp3", bufs=2, space="PSUM") as pp3:
                wo_sb = p2.tile([128, QH, HID], BF16)
                # nb-chunk order: o_proj units are consumed nb-ascending with
                # a one-block delay, so early chunks arrive in time
                for nbc in range(8):
                    nc.sync.dma_start(wo_sb[:, :, nbc * 512:(nbc + 1) * 512],
                                      wo_r[:, :, nbc * 512:(nbc + 1) * 512])

                def oproj_unit(ig, nbc, copy_eng):
                    pso = pp3.tile([128, 512], F32, tag="pso", bufs=2)
                    for c4 in range(QH):
                        nc.tensor.matmul(
                            pso[:],
                            ctxT[:, c4, ig * 128:(ig + 1) * 128],
                            wo_sb[:, c4, nbc * 512:(nbc + 1) * 512],
                            start=(c4 == 0), stop=(c4 == QH - 1))
                    o_sb = p3d.tile([128, 512], BF16, tag="osb")
                    if copy_eng == 0:
                        nc.vector.tensor_copy(o_sb[:], pso[:])
                    else:
                        nc.scalar.activation(o_sb[:], pso[:], AF.Copy)
                    nc.sync.dma_start(
                        out[ig * 128:(ig + 1) * 128,
                            nbc * 512:(nbc + 1) * 512], o_sb[:])

                # pending o_proj units, nb-major within each finished block
                pending = []

                def stage_a(ib, pT_at, take, fillers=()):
                    """scores+exp for all heads of block ib, interleaved with
                    `fillers` thunks first (e.g. the previous block's ctx
                    chains) and then up to `take` pending o_proj units (PE
                    filler while the ACT engine works through the exp
                    chains)."""
                    jmax = 4 * (ib + 1) if causal else NT
                    tiles = [(h, jt) for h in range(QH) for jt in range(jmax)]
                    fillers = list(fillers)
                    k = min(take, len(pending))
                    nfree = max(1, len(tiles) - len(fillers))
                    acc = 0.0
                    for i, (h, jt) in enumerate(tiles):
                        scores_tile(p2s, pp2, pT_at, ib, h, jt, 3)
                        if i < len(fillers):
                            fillers[i]()
                            continue
                        acc += k / nfree
                        while acc >= 1.0 and k > 0:
                            ig, nbc = pending.pop(0)
                            oproj_unit(ig, nbc, 0)
                            acc -= 1.0
                            k -= 1
                    for f in fillers[len(tiles):]:
                        f()
                    while k > 0:
                        ig, nbc = pending.pop(0)
                        oproj_unit(ig, nbc, 0)
                        k -= 1

                def stage_b(ib, pT_at, inline_oproj):
                    """ctx + normalize + transpose + ctxT copy, it-major;
                    optionally (last block) emit o_proj per i-tile inline."""
                    pst = pp2c.tile([128, QH, 128], BF16, tag="pst", bufs=1)
                    prev_units = []
                    for it in range(4):
                        ig = ib * 4 + it
                        jm = ig + 1 if causal else NT
                        cns = []
                        for h in range(QH):
                            psc = pp2c.tile([128, 132], F32, tag="psc",
                                            bufs=2)
                            for jt in range(jm):
                                nc.tensor.matmul(
                                    psc[:, 0:129],
                                    pT_at(h, jt)[:, it * 128:(it + 1) * 128],
                                    vh[:, jt, 0:129],
                                    start=(jt == 0), stop=(jt == jm - 1))
                            rec = p2s.tile([128, 1], F32, tag="rec")
                            nc.vector.reciprocal(rec[:], psc[:, 128:129])
                            cn = p2s.tile([128, 128], BF16, tag="cn")
                            nc.vector.tensor_scalar_mul(cn[:], psc[:, 0:128],
                                                        rec[:])
                            cns.append(cn)
                            # deferred transposes keep >=1 ctx chain between
                            # a cn (DVE) and its transpose (PE)
                            if h >= 1:
                                nc.tensor.transpose(pst[:, h - 1, :],
                                                    cns[h - 1][:], id_sb[:])
                        nc.tensor.transpose(pst[:, QH - 1, :],
                                            cns[QH - 1][:], id_sb[:])
                        nc.scalar.activation(
                            ctxT[:, 0:QH, ig * 128:(ig + 1) * 128],
                            pst[:], AF.Copy)
                        # o_proj for i-tile N emitted during i-tile N+1's ctx
                        # so its first matmul never waits on the ctxT copy
                        for g, nbc in prev_units:
                            oproj_unit(g, nbc, (nbc + 1) % 2)
                        prev_units = ([(ig, nbc) for nbc in range(8)]
                                      if inline_oproj else [])
                    for g, nbc in prev_units:
                        oproj_unit(g, nbc, (nbc + 1) % 2)

                def stage_b_thunks(ib, pT_at):
                    """Block ib's ctx+normalize+transpose+copy chains as
                    thunks, to be emitted as interleave filler inside the
                    NEXT block's stage A (their DVE/ACT latencies then hide
                    behind that stage's matmul stream)."""
                    pst = pp2c.tile([128, QH, 128], BF16, tag="pst", bufs=1)
                    cns = []
                    thunks = []
                    for it in range(4):
                        for h in range(QH):
                            def chain(it=it, h=h):
                                ig = ib * 4 + it
                                jm = ig + 1 if causal else NT
                                psc = pp2c.tile([128, 132], F32, tag="psc",
                                                bufs=2)
                                for jt in range(jm):
                                    nc.tensor.matmul(
                                        psc[:, 0:129],
                                        pT_at(h, jt)[:,
                                                     it * 128:(it + 1) * 128],
                                        vh[:, jt, 0:129],
                                        start=(jt == 0),
                                        stop=(jt == jm - 1))
                                rec = p2s.tile([128, 1], F32, tag="rec")
                                nc.vector.reciprocal(rec[:], psc[:, 128:129])
                                cn = p2s.tile([128, 128], BF16, tag="cn")
                                nc.vector.tensor_scalar_mul(
                                    cn[:], psc[:, 0:128], rec[:])
                                cns.append(cn)
                                if h >= 1:
                                    nc.tensor.transpose(pst[:, h - 1, :],
                                                        cns[-2][:], id_sb[:])
                                if h == QH - 1:
                                    nc.tensor.transpose(pst[:, QH - 1, :],
                                                        cns[-1][:], id_sb[:])
                                    nc.scalar.activation(
                                        ctxT[:, 0:QH,
                                             ig * 128:(ig + 1) * 128],
                                        pst[:], AF.Copy)
                                    cns.clear()
                            thunks.append(chain)
                    return thunks

                # pT4 is split in two halves: the jt<8 half is double-
                # buffered so the NEXT block's early exps can start while
                # this block's ctx chains still read the previous buffer
                # (single-buffer WAR otherwise serializes ACT across blocks)
                def make_pT4(ib):
                    pT4a = p2.tile([128, QH, 8, 512], BF16, tag="pT4a",
                                   bufs=2, name="pT4a")
                    need_b = (not causal) or ib >= 2
                    pT4b = (p2.tile([128, QH, 8, 512], BF16, tag="pT4b",
                                    bufs=1, name="pT4b")
                            if need_b else None)

                    def at(h, jt):
                        if jt < 8:
                            return pT4a[:, h, jt, :]
                        return pT4b[:, h, jt - 8, :]
                    return at

                pT0_at = lambda h, jt: pT0[:, h, jt, :]

                # schedule: A(0) predrained in phase 1 (causal) or first here
                if causal:
                    b0_fill = stage_b_thunks(0, pT0_at)
                else:
                    pT4_at = make_pT4(0)
                    stage_a(0, pT4_at, 0)
                    stage_b(0, pT4_at, False)
                    b0_fill = []
                pending += [(ig, nbc) for nbc in range(8) for ig in range(4)]

                for ib in range(1, NB):
                    pT4_at = make_pT4(ib)
                    take = (24, 36, 36)[ib - 1]
                    stage_a(ib, pT4_at, take,
                            fillers=(b0_fill if ib == 1 else ()))
                    last = ib == NB - 1
                    stage_b(ib, pT4_at, last)
                    if not last:
                        pending += [(4 * ib + ig, nbc)
                                    for nbc in range(8) for ig in range(4)]

    nc.compile()
    return nc


_CACHE: dict = {}


def _get_program(mask_mode: str):
    if mask_mode not in _CACHE:
        _CACHE[mask_mode] = build_program(mask_mode)
    return _CACHE[mask_mode]


def _host_tensors():
    """Position-dependent constants shared by every call."""
    inv_freq = 1.0 / (THETA ** (np.arange(0, HD, 2, dtype=np.float32) / HD))
    t = np.arange(S, dtype=np.float32)
    freqs = np.outer(t, inv_freq)                     # [S, 64]
    emb = np.concatenate([freqs, freqs], axis=-1)     # [S, 128]
    cosT = np.cos(emb).T.astype(np.float32).copy()    # [128, S]
    sinT = np.sin(emb).T.astype(np.float32).copy()
    sinT[0:64] *= -1.0                                # fold rotate_half sign
    cs = np.ascontiguousarray(np.stack([cosT, sinT])).astype(bfloat16)
    idm = np.eye(128, dtype=np.float32).astype(bfloat16)
    jj = np.arange(128)[:, None]
    ii = np.arange(128)[None, :]
    tri = np.where(ii >= jj, 1.0, 0.0).astype(bfloat16)  # [128, 128]
    return cs, idm, tri


def _in_maps(hidden_states, Wq, Wk, Wv, Wo, mask, mode):
    cs, idm, tri = _host_tensors()
    hT = np.ascontiguousarray(hidden_states[0].T).astype(bfloat16)
    wq_b = Wq.astype(bfloat16)
    wk_b = Wk.astype(bfloat16)
    wv_b = Wv.astype(bfloat16)
    wo_b = Wo.astype(bfloat16)
    in_maps = []
    for c in range(NCORES):
        m = {
            "hT": hT,
            "wq": np.ascontiguousarray(wq_b[:, c * DQ:(c + 1) * DQ]),
            "wk": np.ascontiguousarray(wk_b[:, c * HD:(c + 1) * HD]),
            "wv": np.ascontiguousarray(wv_b[:, c * HD:(c + 1) * HD]),
            "wo": np.ascontiguousarray(wo_b[c * DQ:(c + 1) * DQ, :]),
            "cs": cs,
            "idm": idm,
        }
        if mode == "causal":
            m["tri"] = tri
        if mode == "full":
            m["maskT"] = np.ascontiguousarray(mask.T * math.sqrt(HD)).astype(
                np.float32)
        in_maps.append(m)
    return in_maps


def kernel(hidden_states, Wq, Wk, Wv, Wo, attention_mask):
    hidden_states = np.asarray(hidden_states, dtype=np.float32)
    Wq = np.asarray(Wq, dtype=np.float32)
    Wk = np.asarray(Wk, dtype=np.float32)
    Wv = np.asarray(Wv, dtype=np.float32)
    Wo = np.asarray(Wo, dtype=np.float32)
    mask = np.asarray(attention_mask, dtype=np.float32)[0, 0]

    causal_ref = np.triu(np.full((S, S), NEG, dtype=np.float32), k=1)
    if np.array_equal(mask, causal_ref):
        mode = "causal"
    elif not mask.any():
        mode = "none"
    else:
        mode = "full"

    nc = _get_program(mode)
    in_maps = _in_maps(hidden_states, Wq, Wk, Wv, Wo, mask, mode)
    res = run_bass_kernel_spmd(nc, in_maps, core_ids=list(range(NCORES)))
    total = res.results[0]["out"].astype(np.float32)
    for c in range(1, NCORES):
        total = total + res.results[c]["out"].astype(np.float32)
    return total.reshape(B, S, HID).astype(np.float32)
